# revision 6
# baseline (speedup 1.0000x reference)
"""Self-contained Trainium2 Bass kernel for GQA attention (RoPE + causal).

Problem: hidden (B=2, S=2048, HID=2048), W_qkv (3072, 2048) = 16 Q heads +
2*4 KV heads of dim 128, RoPE, causal GQA attention, W_o (2048, 2048).

Sharding: 8 cores = (batch b in {0,1}) x (KV group g in {0..3}).  Each core
gets 4 Q heads + 1 KV head (the GQA group stays intact), computes its
partial output through the 512 matching W_o columns, and the host sums the
4 partials per batch (the unshard step).  No on-device collectives.

Layout: everything "transposed" (feature dim on SBUF partitions) so every
matmul contraction lands on the partition axis.  v2 schedule:
  - the softmax denominator is accumulated on the Vector engine (f32) and
    reduced by ONE small ones-matmul per chunk instead of a full-width
    ones-matmul per t-tile (removes ~57k PE columns).
  - the causal mask is applied additively in PSUM by a tiny accumulating
    matmul (maskAdd @ I) between the scores matmul and the exp, instead of
    a GpSimd multiply on the exp output.
  - attention for head i is interleaved (in PE program order) with the
    QKV projection of head i+1, and attention of the last head with the
    W_o projection, so the ScalarE exp stream hides under matmuls.
  - DMA order: kv-weight slice + hidden tile per contraction step first
    (the host lays W out as [wk|wv|wq]), q weights after, W_o during
    attention.
Matmul operands are bf16; accumulation/softmax arithmetic stays f32.
"""

import math

import numpy as np
import ml_dtypes

import concourse.bass as bass
import concourse.bacc as bacc
import concourse.mybir as mybir
from concourse.tile import TileContext
from concourse.bass_utils import run_bass_kernel_spmd

F32 = mybir.dt.float32
BF16 = mybir.dt.bfloat16
AF = mybir.ActivationFunctionType

P = 128  # SBUF partitions / head dim / tile edge


def build_attn_nc(S=2048, HID=2048, NQ=4, HD=128, SC=512):
    """One NeuronCore graph: NQ query heads + 1 KV head, full sequence."""
    assert HD == P
    n_h = HID // P   # contraction tiles of the QKV projection
    n_st = S // P    # 128-row tiles of the sequence
    n_sc = S // SC   # SC-wide chunks of the sequence
    n_tc = SC // P   # t-tiles per sq chunk
    n_ec = HID // SC
    scale = 1.0 / math.sqrt(HD)

    nc = bacc.Bacc("TRN2", target_bir_lowering=False, debug=False, num_devices=8)
    hidT = nc.declare_dram_parameter("hidT", [HID, S], BF16, isOutput=False)
    # host layout: [wk | wv | wq0..wq3] so the kv slice loads first
    wqkvT = nc.declare_dram_parameter("wqkvT", [HID, (NQ + 2) * P], BF16,
                                      isOutput=False)
    woT = nc.declare_dram_parameter("woT", [NQ * HD, HID], BF16, isOutput=False)
    cosT = nc.declare_dram_parameter("cosT", [HD, S], BF16, isOutput=False)
    sinT = nc.declare_dram_parameter("sinT", [HD, S], BF16, isOutput=False)
    rT = nc.declare_dram_parameter("rT", [HD, HD], BF16, isOutput=False)
    ident = nc.declare_dram_parameter("ident", [P, P], BF16, isOutput=False)
    maskA = nc.declare_dram_parameter("maskA", [P, P], BF16, isOutput=False)
    out = nc.declare_dram_parameter("out", [S, HID], BF16, isOutput=True)

    with TileContext(nc) as tc:
        with (
            tc.tile_pool(name="const", bufs=1) as constp,
            tc.tile_pool(name="wbf", bufs=1) as wbfp,
            tc.tile_pool(name="big", bufs=1) as bigp,
            tc.tile_pool(name="raw", bufs=4) as rawp,
            tc.tile_pool(name="act", bufs=1) as actp,
            tc.tile_pool(name="es", bufs=6) as esp,
            tc.tile_pool(name="accs", bufs=2) as accsp,
            tc.tile_pool(name="outsb", bufs=4) as outp,
            tc.tile_pool(name="ps", bufs=1, space="PSUM") as psp,
        ):
            # ---------------- DMA: kv weights + hidden first --------------
            w_bf, hid_bf = [], []
            for h in range(n_h):
                wb = wbfp.tile([P, (NQ + 2) * P], BF16, tag=f"w{h}",
                               name=f"wb{h}")
                nc.sync.dma_start(out=wb[:, :2 * P],
                                  in_=wqkvT[h * P:(h + 1) * P, :2 * P])
                w_bf.append(wb)
                hb = bigp.tile([P, S], BF16, tag=f"hid{h}", name=f"hb{h}")
                pieces = 4 if h < 2 else 2
                step = S // pieces
                for pc in range(pieces):
                    nc.sync.dma_start(
                        out=hb[:, pc * step:(pc + 1) * step],
                        in_=hidT[h * P:(h + 1) * P, pc * step:(pc + 1) * step])
                hid_bf.append(hb)
            # q weights (needed from the q0 pass onwards)
            for h in range(n_h):
                nc.sync.dma_start(out=w_bf[h][:, 2 * P:],
                                  in_=wqkvT[h * P:(h + 1) * P, 2 * P:])
            rT_sb = constp.tile([HD, HD], BF16, name="rT_sb")
            nc.sync.dma_start(out=rT_sb[:, :], in_=rT[:, :])
            id_sb = constp.tile([P, P], BF16, name="id_sb")
            nc.sync.dma_start(out=id_sb[:, :], in_=ident[:, :])
            mask_sb = constp.tile([P, P], BF16, name="mask_sb")
            nc.sync.dma_start(out=mask_sb[:, :], in_=maskA[:, :])
            cos_sb = constp.tile([HD, S], BF16, name="cos_sb")
            nc.sync.dma_start(out=cos_sb[:, :], in_=cosT[:, :])
            sin_sb = constp.tile([HD, S], BF16, name="sin_sb")
            nc.sync.dma_start(out=sin_sb[:, :], in_=sinT[:, :])
            wo_bf = []
            for hh in range(NQ):
                wob = bigp.tile([P, HID], BF16, tag=f"wo{hh}", name=f"wob{hh}")
                nc.sync.dma_start(out=wob[:, :HID // 2],
                                  in_=woT[hh * P:(hh + 1) * P, :HID // 2])
                nc.sync.dma_start(out=wob[:, HID // 2:],
                                  in_=woT[hh * P:(hh + 1) * P, HID // 2:])
                wo_bf.append(wob)
            ones_sq = constp.tile([P, P], BF16, name="ones_sq")
            nc.gpsimd.memset(ones_sq[:, :], 1.0)

            qhat = [actp.tile([HD, S], BF16, tag=f"qhat{i}", name=f"qhat{i}")
                    for i in range(NQ)]
            khat = actp.tile([HD, S], BF16, tag="khat", name="khat")
            V_bf = actp.tile([P, S], BF16, tag="V", name="V_bf")
            ohat = [actp.tile([HD, S], BF16, tag=f"ohat{i}", name=f"ohat{i}")
                    for i in range(NQ)]

            # ---------------- kv pass (all 8 PSUM banks) ------------------
            ps_k = [psp.tile([P, SC], F32, tag="A", name=f"psk{c}")
                    for c in range(n_sc)]
            ps_v = [psp.tile([P, SC], F32, tag="s" if c < 2 else "o",
                             name=f"psv{c}") for c in range(n_sc)]
            for h in range(n_h):
                for c in range(n_sc):
                    nc.tensor.matmul(ps_k[c][:, :], lhsT=w_bf[h][:, :P],
                                     rhs=hid_bf[h][:, c * SC:(c + 1) * SC],
                                     start=(h == 0), stop=(h == n_h - 1))
                    nc.tensor.matmul(ps_v[c][:, :], lhsT=w_bf[h][:, P:2 * P],
                                     rhs=hid_bf[h][:, c * SC:(c + 1) * SC],
                                     start=(h == 0), stop=(h == n_h - 1))

            def rope_chunk(ps, c, dest):
                """psum chunk -> raw copy -> rotate matmul -> dest (roped)."""
                csl = slice(c * SC, (c + 1) * SC)
                raw = rawp.tile([P, SC], BF16, tag="raw", name=f"rw{c}")
                nc.scalar.copy(out=raw[:, :], in_=ps[:, :])
                psr = psp.tile([P, SC], F32, tag="A", name=f"psr{c}")
                nc.tensor.matmul(psr[:, :], lhsT=rT_sb[:, :], rhs=raw[:, :],
                                 start=True, stop=True)
                t1 = rawp.tile([P, SC], BF16, tag="t1", name=f"t1_{c}")
                nc.vector.tensor_mul(t1[:, :], psr[:, :], sin_sb[:, csl])
                t2 = rawp.tile([P, SC], BF16, tag="t2", name=f"t2_{c}")
                nc.vector.tensor_mul(t2[:, :], raw[:, :], cos_sb[:, csl])
                nc.vector.tensor_add(dest[:, csl], t1[:, :], t2[:, :])

            # v: copy + transpose to natural (t, d) layout
            rawv = rawp.tile([P, S], BF16, tag="rawv", name="rawv", bufs=1)
            for c in range(n_sc):
                nc.vector.tensor_copy(rawv[:, c * SC:(c + 1) * SC],
                                      ps_v[c][:, :])
            # k: rope into khat (frees ps_k banks for q0)
            for c in range(n_sc):
                rope_chunk(ps_k[c], c, khat)
            for st in range(n_st):
                pst = psp.tile([P, P], BF16, tag="s", name=f"pvt{st}")
                nc.tensor.transpose(pst[:, :], rawv[:, st * P:(st + 1) * P],
                                    id_sb[:, :])
                nc.vector.tensor_copy(V_bf[:, st * P:(st + 1) * P], pst[:, :])

            # ---------------- emitters for interleaved streams ------------
            def proj_units(o):
                """Closures: q-head o projection + rope, chunk-major
                (reversed chunk order so attention can follow in order)."""
                units = []
                for c in reversed(range(n_sc)):
                    ps = psp.tile([P, SC], F32, tag="A", name=f"pq{o}_{c}")
                    for h in range(n_h):
                        units.append(lambda ps=ps, h=h, c=c, o=o: nc.tensor.matmul(
                            ps[:, :], lhsT=w_bf[h][:, (2 + o) * P:(3 + o) * P],
                            rhs=hid_bf[h][:, c * SC:(c + 1) * SC],
                            start=(h == 0), stop=(h == n_h - 1)))
                    units.append(lambda ps=ps, c=c, o=o: rope_chunk(
                        ps, c, qhat[o]))
                return units

            def wo_units(sc):
                """Closures: W_o projection for the n_tc s-tiles of chunk sc
                + output DMA."""
                units = []
                for st in range(sc * n_tc, (sc + 1) * n_tc):
                    for ec in range(n_ec):
                        po = psp.tile([P, SC], F32, tag="A",
                                      name=f"pw{st}_{ec}")
                        for hh in range(NQ):
                            units.append(lambda po=po, hh=hh, st=st, ec=ec:
                                nc.tensor.matmul(
                                    po[:, :],
                                    lhsT=ohat[hh][:, st * P:(st + 1) * P],
                                    rhs=wo_bf[hh][:, ec * SC:(ec + 1) * SC],
                                    start=(hh == 0), stop=(hh == NQ - 1)))

                        def fin(po=po, st=st, ec=ec):
                            ot = outp.tile([P, SC], BF16, tag="osb",
                                           name=f"osb{st}_{ec}")
                            if ec % 2 == 0:
                                nc.scalar.copy(out=ot[:, :], in_=po[:, :])
                            else:
                                nc.vector.tensor_copy(ot[:, :], po[:, :])
                            nc.sync.dma_start(
                                out=out[st * P:(st + 1) * P,
                                        ec * SC:(ec + 1) * SC],
                                in_=ot[:, :])
                        units.append(fin)
                return units

            def attn_head(q, on_chunk_done):
                """Generator: causal attention for head q, chunk-major
                (big chunks first).  Yields after each t-tile so the caller
                can interleave other PE work."""
                for sc in reversed(range(n_sc)):
                    csl = slice(sc * SC, (sc + 1) * SC)
                    n_t = n_tc * (sc + 1)
                    ps_o = psp.tile([HD, SC], F32, tag="o", name=f"po{q}_{sc}")
                    acc = accsp.tile([P, SC], F32, tag="acc",
                                     name=f"acc{q}_{sc}")
                    for tt in range(n_t):
                        j = tt - n_tc * sc
                        c0 = j * P if j > 0 else 0
                        ps_s = psp.tile([P, SC], F32, tag="s",
                                        name=f"pss{q}_{sc}_{tt}")
                        if j >= 0:
                            nc.tensor.matmul(
                                ps_s[:, c0:],
                                lhsT=khat[:, tt * P:(tt + 1) * P],
                                rhs=qhat[q][:, sc * SC + c0:(sc + 1) * SC],
                                start=True, stop=False)
                            nc.tensor.matmul(
                                ps_s[:, c0:c0 + P], lhsT=mask_sb[:, :],
                                rhs=id_sb[:, :], start=False, stop=True,
                                skip_group_check=True)
                        else:
                            nc.tensor.matmul(
                                ps_s[:, :],
                                lhsT=khat[:, tt * P:(tt + 1) * P],
                                rhs=qhat[q][:, csl], start=True, stop=True)
                        es = esp.tile([P, SC], BF16, tag="es",
                                      name=f"es{q}_{sc}_{tt}")
                        nc.scalar.activation(es[:, c0:], ps_s[:, c0:], AF.Exp,
                                             scale=scale)
                        if tt == 0:
                            nc.vector.tensor_copy(acc[:, :], es[:, :])
                        else:
                            nc.vector.tensor_add(acc[:, c0:], acc[:, c0:],
                                                 es[:, c0:])
                        nc.tensor.matmul(ps_o[:, c0:],
                                         lhsT=V_bf[:, tt * P:(tt + 1) * P],
                                         rhs=es[:, c0:],
                                         start=(tt == 0), stop=(tt == n_t - 1))
                        yield
                    accb = accsp.tile([P, SC], BF16, tag="accb",
                                      name=f"accb{q}_{sc}")
                    nc.vector.tensor_copy(accb[:, :], acc[:, :])
                    ps_r = psp.tile([P, SC], F32, tag="s", name=f"pr{q}_{sc}")
                    nc.tensor.matmul(ps_r[:, :], lhsT=ones_sq[:, :],
                                     rhs=accb[:, :], start=True, stop=True)
                    rr = accsp.tile([P, SC], F32, tag="rr", name=f"rr{q}_{sc}")
                    nc.vector.reciprocal_approx_fast(out=rr[:, :],
                                                     in_=ps_r[:, :])
                    nc.vector.tensor_mul(ohat[q][:, csl], ps_o[:, :], rr[:, :])
                    on_chunk_done(sc)
                    yield

            def run_interleaved(gen, units):
                """Drive the attention generator, pacing `units` closures
                evenly across its yields."""
                done = 0
                steps = 0
                total_steps = n_sc + n_tc * n_sc * (n_sc + 1) // 2
                for _ in gen:
                    steps += 1
                    want = len(units) * steps // total_steps
                    while done < want:
                        units[done]()
                        done += 1
                while done < len(units):
                    units[done]()
                    done += 1

            # ---------------- q0 projection (straight) --------------------
            for u in proj_units(0):
                u()

            # ---------------- heads 0..2: attention || next projection ----
            for i in range(NQ - 1):
                run_interleaved(attn_head(i, lambda sc: None),
                                proj_units(i + 1))

            # ---------------- last head: attention || W_o ------------------
            pending_wo = []

            def on_chunk(sc):
                pending_wo.extend(wo_units(sc))

            gen = attn_head(NQ - 1, on_chunk)
            total_steps = n_sc + n_tc * n_sc * (n_sc + 1) // 2
            steps = 0
            done = 0
            for _ in gen:
                steps += 1
                # drain Wo work at a rate that leaves the tail short:
                # after each yield run up to 10 pending units
                budget = 10
                while pending_wo[done:] and budget > 0:
                    pending_wo[done]()
                    done += 1
                    budget -= 1
            while done < len(pending_wo):
                pending_wo[done]()
                done += 1
    nc.compile()
    return nc


def make_host_constants(S, HD=128, SC=512):
    rt = np.zeros((HD, HD), np.float32)
    half = HD // 2
    for j in range(half):
        rt[j, j + half] = 1.0       # R^T upper-right block = +I
        rt[j + half, j] = -1.0      # R^T lower-left block = -I
    ident = np.eye(P, dtype=np.float32)
    a_idx = np.arange(P)[:, None]
    b_idx = np.arange(P)[None, :]
    mask_add = np.where(b_idx > a_idx, -1e9, 0.0).astype(np.float32)
    bf = ml_dtypes.bfloat16
    return rt.astype(bf), ident.astype(bf), mask_add.astype(bf)


def make_in_maps(hidden_states, cos, sin, W_qkv, W_o, NH=16, NKV=4, HD=128):
    """Shard the full inputs into 8 per-core input maps."""
    B = hidden_states.shape[0]
    S = hidden_states.shape[1]
    n_rep = NH // NKV
    rt, ident, mask_add = make_host_constants(S, HD)
    bf = ml_dtypes.bfloat16
    cosT = np.ascontiguousarray(cos.T).astype(bf)
    sinT = np.ascontiguousarray(sin.T).astype(bf)
    in_maps = []
    for b in range(B):
        hidT = np.ascontiguousarray(hidden_states[b].T).astype(bf)
        for g in range(NKV):
            wq = W_qkv[g * n_rep * HD:(g + 1) * n_rep * HD]
            wk = W_qkv[NH * HD + g * HD: NH * HD + (g + 1) * HD]
            wv = W_qkv[(NH + NKV) * HD + g * HD: (NH + NKV) * HD + (g + 1) * HD]
            wsh = np.concatenate([wk, wv, wq], axis=0)
            wqkvT = np.ascontiguousarray(wsh.T).astype(bf)
            woT = np.ascontiguousarray(
                W_o[:, g * n_rep * HD:(g + 1) * n_rep * HD].T).astype(bf)
            in_maps.append({
                "hidT": hidT, "wqkvT": wqkvT, "woT": woT,
                "cosT": cosT, "sinT": sinT,
                "rT": rt, "ident": ident, "maskA": mask_add,
            })
    return in_maps


_NC_CACHE = {}


def kernel(hidden_states, cos, sin, W_qkv, W_o):
    hidden_states = np.asarray(hidden_states, dtype=np.float32)
    cos = np.asarray(cos, dtype=np.float32)
    sin = np.asarray(sin, dtype=np.float32)
    W_qkv = np.asarray(W_qkv, dtype=np.float32)
    W_o = np.asarray(W_o, dtype=np.float32)

    B, S, HID = hidden_states.shape
    HD = cos.shape[-1]
    NH = W_o.shape[1] // HD
    NKV = (W_qkv.shape[0] // HD - NH) // 2
    n_rep = NH // NKV

    key = (S, HID, n_rep, HD)
    if key not in _NC_CACHE:
        _NC_CACHE[key] = build_attn_nc(S=S, HID=HID, NQ=n_rep, HD=HD)
    nc = _NC_CACHE[key]

    in_maps = make_in_maps(hidden_states, cos, sin, W_qkv, W_o, NH, NKV, HD)
    res = run_bass_kernel_spmd(nc, in_maps, core_ids=list(range(B * NKV)))
    outs = [np.asarray(r["out"], dtype=np.float32) for r in res.results]
    full = np.stack(
        [np.sum(outs[b * NKV:(b + 1) * NKV], axis=0, dtype=np.float32)
         for b in range(B)], axis=0)
    return full.astype(np.float32)


# revision 7
# speedup vs baseline: 1.1980x; 1.1980x over previous
"""Self-contained Trainium2 Bass kernel for GQA attention (RoPE + causal).

Problem: hidden (B=2, S=2048, HID=2048), W_qkv (3072, 2048) = 16 Q heads +
2*4 KV heads of dim 128, RoPE, causal GQA attention, W_o (2048, 2048).

Sharding: 8 cores = (batch b in {0,1}) x (KV group g in {0..3}).  Each core
gets 4 Q heads + 1 KV head (the GQA group stays intact), computes its
partial output through the 512 matching W_o columns, and the host sums the
4 partials per batch (the unshard step).  No on-device collectives.

Layout: everything "transposed" (feature dim on SBUF partitions) so every
matmul contraction lands on the partition axis.  v3 schedule:
  - attention per head is software-pipelined one t-tile deep (PV/rowsum of
    tile tt are emitted after scores of tile tt+1) so the PE never sits in
    the scores->exp->PV latency chain,
  - attention for head i is interleaved (in PE program order) with the QKV
    projection of head i+1, and attention of the last head with the W_o
    projection, so the ScalarE exp stream hides under matmuls,
  - ScalarE runs ONLY the exp stream; psum->sbuf copies live on the Vector
    engine, the causal mask multiply on GpSimd,
  - DMA order: kv-weight slice + hidden tile per contraction step first
    (the host lays W out as [wk|wv|wq]), q weights after, W_o during
    attention.
Matmul operands are bf16; accumulation/softmax arithmetic stays f32.
"""

import math

import numpy as np
import ml_dtypes

import concourse.bass as bass
import concourse.bacc as bacc
import concourse.mybir as mybir
from concourse.tile import TileContext
from concourse.bass_utils import run_bass_kernel_spmd

F32 = mybir.dt.float32
BF16 = mybir.dt.bfloat16
AF = mybir.ActivationFunctionType

P = 128  # SBUF partitions / head dim / tile edge


def build_attn_nc(S=2048, HID=2048, NQ=4, HD=128, SC=512):
    """One NeuronCore graph: NQ query heads + 1 KV head, full sequence."""
    assert HD == P
    n_h = HID // P   # contraction tiles of the QKV projection
    n_st = S // P    # 128-row tiles of the sequence
    n_sc = S // SC   # SC-wide chunks of the sequence
    n_tc = SC // P   # t-tiles per sq chunk
    n_ec = HID // SC
    scale = 1.0 / math.sqrt(HD)

    nc = bacc.Bacc("TRN2", target_bir_lowering=False, debug=False, num_devices=8)
    hidT = nc.declare_dram_parameter("hidT", [HID, S], BF16, isOutput=False)
    # host layout: [wk | wv | wq0..wq3] so the kv slice loads first
    wqkvT = nc.declare_dram_parameter("wqkvT", [HID, (NQ + 2) * P], BF16,
                                      isOutput=False)
    woT = nc.declare_dram_parameter("woT", [NQ * HD, HID], BF16, isOutput=False)
    cosT = nc.declare_dram_parameter("cosT", [HD, S], BF16, isOutput=False)
    sinT = nc.declare_dram_parameter("sinT", [HD, S], BF16, isOutput=False)
    rT = nc.declare_dram_parameter("rT", [HD, HD], BF16, isOutput=False)
    ident = nc.declare_dram_parameter("ident", [P, P], BF16, isOutput=False)
    maskM = nc.declare_dram_parameter("maskM", [P, P], BF16, isOutput=False)
    out = nc.declare_dram_parameter("out", [S, HID], BF16, isOutput=True)

    with TileContext(nc) as tc:
        with (
            tc.tile_pool(name="const", bufs=1) as constp,
            tc.tile_pool(name="wbf", bufs=1) as wbfp,
            tc.tile_pool(name="big", bufs=1) as bigp,
            tc.tile_pool(name="raw", bufs=4) as rawp,
            tc.tile_pool(name="act", bufs=1) as actp,
            tc.tile_pool(name="es", bufs=12) as esp,
            tc.tile_pool(name="rrp", bufs=2) as rrp,
            tc.tile_pool(name="outsb", bufs=4) as outp,
            tc.tile_pool(name="ps", bufs=1, space="PSUM") as psp,
        ):
            # ---------------- DMA: kv weights + hidden first --------------
            w_bf, hid_bf = [], []
            for h in range(n_h):
                wb = wbfp.tile([P, (NQ + 2) * P], BF16, tag=f"w{h}",
                               name=f"wb{h}")
                nc.sync.dma_start(out=wb[:, :2 * P],
                                  in_=wqkvT[h * P:(h + 1) * P, :2 * P])
                w_bf.append(wb)
                hb = bigp.tile([P, S], BF16, tag=f"hid{h}", name=f"hb{h}")
                pieces = 4 if h < 2 else 2
                step = S // pieces
                for pc in range(pieces):
                    nc.sync.dma_start(
                        out=hb[:, pc * step:(pc + 1) * step],
                        in_=hidT[h * P:(h + 1) * P, pc * step:(pc + 1) * step])
                hid_bf.append(hb)
            # q weights (needed from the q0 pass onwards)
            for h in range(n_h):
                nc.sync.dma_start(out=w_bf[h][:, 2 * P:],
                                  in_=wqkvT[h * P:(h + 1) * P, 2 * P:])
            rT_sb = constp.tile([HD, HD], BF16, name="rT_sb")
            nc.sync.dma_start(out=rT_sb[:, :], in_=rT[:, :])
            id_sb = constp.tile([P, P], BF16, name="id_sb")
            nc.sync.dma_start(out=id_sb[:, :], in_=ident[:, :])
            mask_sb = constp.tile([P, P], BF16, name="mask_sb")
            nc.sync.dma_start(out=mask_sb[:, :], in_=maskM[:, :])
            cos_sb = constp.tile([HD, S], BF16, name="cos_sb")
            nc.sync.dma_start(out=cos_sb[:, :], in_=cosT[:, :])
            sin_sb = constp.tile([HD, S], BF16, name="sin_sb")
            nc.sync.dma_start(out=sin_sb[:, :], in_=sinT[:, :])
            wo_bf = []
            for hh in range(NQ):
                wob = bigp.tile([P, HID], BF16, tag=f"wo{hh}", name=f"wob{hh}")
                nc.sync.dma_start(out=wob[:, :HID // 2],
                                  in_=woT[hh * P:(hh + 1) * P, :HID // 2])
                nc.sync.dma_start(out=wob[:, HID // 2:],
                                  in_=woT[hh * P:(hh + 1) * P, HID // 2:])
                wo_bf.append(wob)
            ones_sq = constp.tile([P, P], BF16, name="ones_sq")
            nc.gpsimd.memset(ones_sq[:, :], 1.0)

            qhat = [actp.tile([HD, S], BF16, tag=f"qhat{i}", name=f"qhat{i}")
                    for i in range(NQ)]
            khat = actp.tile([HD, S], BF16, tag="khat", name="khat")
            V_bf = actp.tile([P, S], BF16, tag="V", name="V_bf")
            ohat = [actp.tile([HD, S], BF16, tag=f"ohat{i}", name=f"ohat{i}")
                    for i in range(NQ)]

            # ---------------- kv pass (all 8 PSUM banks) ------------------
            # psum tags: A=3, s=2, o=2, r=1  (8 banks total)
            ktags = ["A", "A", "A", "r"]
            vtags = ["s", "s", "o", "o"]
            ps_k = [psp.tile([P, SC], F32, tag=ktags[c], name=f"psk{c}")
                    for c in range(n_sc)]
            ps_v = [psp.tile([P, SC], F32, tag=vtags[c], name=f"psv{c}")
                    for c in range(n_sc)]
            for h in range(n_h):
                for c in range(n_sc):
                    nc.tensor.matmul(ps_k[c][:, :], lhsT=w_bf[h][:, :P],
                                     rhs=hid_bf[h][:, c * SC:(c + 1) * SC],
                                     start=(h == 0), stop=(h == n_h - 1))
                    nc.tensor.matmul(ps_v[c][:, :], lhsT=w_bf[h][:, P:2 * P],
                                     rhs=hid_bf[h][:, c * SC:(c + 1) * SC],
                                     start=(h == 0), stop=(h == n_h - 1))

            def rope_chunk(ps, c, dest):
                """psum chunk -> raw copy -> rotate matmul -> dest (roped)."""
                csl = slice(c * SC, (c + 1) * SC)
                raw = rawp.tile([P, SC], BF16, tag="raw", name=f"rw{c}")
                nc.vector.tensor_copy(raw[:, :], ps[:, :])
                psr = psp.tile([P, SC], F32, tag="A", name=f"psr{c}")
                nc.tensor.matmul(psr[:, :], lhsT=rT_sb[:, :], rhs=raw[:, :],
                                 start=True, stop=True)
                t1 = rawp.tile([P, SC], BF16, tag="t1", name=f"t1_{c}")
                nc.vector.tensor_mul(t1[:, :], psr[:, :], sin_sb[:, csl])
                t2 = rawp.tile([P, SC], BF16, tag="t2", name=f"t2_{c}")
                nc.vector.tensor_mul(t2[:, :], raw[:, :], cos_sb[:, csl])
                nc.vector.tensor_add(dest[:, csl], t1[:, :], t2[:, :])

            # v: copy + transpose to natural (t, d) layout
            rawv = rawp.tile([P, S], BF16, tag="rawv", name="rawv", bufs=1)
            for c in range(n_sc):
                nc.vector.tensor_copy(rawv[:, c * SC:(c + 1) * SC],
                                      ps_v[c][:, :])
            # k: rope into khat (frees psum banks for q0)
            for c in range(n_sc):
                rope_chunk(ps_k[c], c, khat)
            for st in range(n_st):
                pst = psp.tile([P, P], BF16, tag="s", name=f"pvt{st}")
                nc.tensor.transpose(pst[:, :], rawv[:, st * P:(st + 1) * P],
                                    id_sb[:, :])
                nc.vector.tensor_copy(V_bf[:, st * P:(st + 1) * P], pst[:, :])

            # ---------------- emitters for interleaved streams ------------
            def proj_units(o):
                """Closures: q-head o projection + rope, chunk-major
                (reversed chunk order so attention can follow in order)."""
                units = []
                for c in reversed(range(n_sc)):
                    ps = psp.tile([P, SC], F32, tag="A", name=f"pq{o}_{c}")
                    for h in range(n_h):
                        units.append(lambda ps=ps, h=h, c=c, o=o: nc.tensor.matmul(
                            ps[:, :], lhsT=w_bf[h][:, (2 + o) * P:(3 + o) * P],
                            rhs=hid_bf[h][:, c * SC:(c + 1) * SC],
                            start=(h == 0), stop=(h == n_h - 1)))
                    units.append(lambda ps=ps, c=c, o=o: rope_chunk(
                        ps, c, qhat[o]))
                return units

            def wo_units(sc):
                """Closures: W_o projection for the n_tc s-tiles of chunk sc
                + output DMA."""
                units = []
                for st in range(sc * n_tc, (sc + 1) * n_tc):
                    for ec in range(n_ec):
                        po = psp.tile([P, SC], F32, tag="A",
                                      name=f"pw{st}_{ec}")
                        for hh in range(NQ):
                            units.append(lambda po=po, hh=hh, st=st, ec=ec:
                                nc.tensor.matmul(
                                    po[:, :],
                                    lhsT=ohat[hh][:, st * P:(st + 1) * P],
                                    rhs=wo_bf[hh][:, ec * SC:(ec + 1) * SC],
                                    start=(hh == 0), stop=(hh == NQ - 1)))

                        def fin(po=po, st=st, ec=ec):
                            ot = outp.tile([P, SC], BF16, tag="osb",
                                           name=f"osb{st}_{ec}")
                            if ec % 2 == 0:
                                nc.scalar.copy(out=ot[:, :], in_=po[:, :])
                            else:
                                nc.vector.tensor_copy(ot[:, :], po[:, :])
                            nc.sync.dma_start(
                                out=out[st * P:(st + 1) * P,
                                        ec * SC:(ec + 1) * SC],
                                in_=ot[:, :])
                        units.append(fin)
                return units

            def attn_head(q, on_chunk_done):
                """Generator: causal attention for head q, chunk-major (big
                chunks first), software-pipelined one t-tile deep: PV/rowsum
                of tile tt are emitted after scores/exp of tile tt+1.
                Yields after each t-tile so the caller can interleave other
                PE work into the exp latency."""
                for sc in reversed(range(n_sc)):
                    csl = slice(sc * SC, (sc + 1) * SC)
                    n_t = n_tc * (sc + 1)
                    ps_o = psp.tile([HD, SC], F32, tag="o", name=f"po{q}_{sc}")
                    ps_r = psp.tile([P, SC], F32, tag="r", name=f"pr{q}_{sc}")
                    prev = None

                    def pv_rowsum(tt, c0, es, last):
                        nc.tensor.matmul(ps_o[:, c0:],
                                         lhsT=V_bf[:, tt * P:(tt + 1) * P],
                                         rhs=es[:, c0:],
                                         start=(tt == 0), stop=last)
                        nc.tensor.matmul(ps_r[:, c0:], lhsT=ones_sq[:, :],
                                         rhs=es[:, c0:],
                                         start=(tt == 0), stop=last)

                    for tt in range(n_t):
                        j = tt - n_tc * sc
                        c0 = j * P if j > 0 else 0
                        ps_s = psp.tile([P, SC], F32, tag="s",
                                        name=f"pss{q}_{sc}_{tt}")
                        nc.tensor.matmul(
                            ps_s[:, c0:],
                            lhsT=khat[:, tt * P:(tt + 1) * P],
                            rhs=qhat[q][:, sc * SC + c0:(sc + 1) * SC],
                            start=True, stop=True)
                        es = esp.tile([P, SC], BF16, tag="es",
                                      name=f"es{q}_{sc}_{tt}")
                        nc.scalar.activation(es[:, c0:], ps_s[:, c0:], AF.Exp,
                                             scale=scale)
                        if j >= 0:
                            nc.gpsimd.tensor_mul(es[:, c0:c0 + P],
                                                 es[:, c0:c0 + P],
                                                 mask_sb[:, :])
                        if prev is not None:
                            pv_rowsum(*prev, last=False)
                        prev = (tt, c0, es)
                        yield
                    pv_rowsum(*prev, last=True)
                    rr = rrp.tile([P, SC], F32, tag="rr", name=f"rr{q}_{sc}")
                    nc.vector.reciprocal_approx_fast(out=rr[:, :],
                                                     in_=ps_r[:, :])
                    nc.vector.tensor_mul(ohat[q][:, csl], ps_o[:, :], rr[:, :])
                    on_chunk_done(sc)
                    yield

            def run_interleaved(gen, units):
                """Drive the attention generator, pacing `units` closures
                evenly across its yields."""
                done = 0
                steps = 0
                total_steps = n_sc + n_tc * n_sc * (n_sc + 1) // 2
                for _ in gen:
                    steps += 1
                    want = len(units) * steps // total_steps
                    while done < want:
                        units[done]()
                        done += 1
                while done < len(units):
                    units[done]()
                    done += 1

            # ---------------- q0 projection (straight) --------------------
            for u in proj_units(0):
                u()

            # ---------------- heads 0..2: attention || next projection ----
            for i in range(NQ - 1):
                run_interleaved(attn_head(i, lambda sc: None),
                                proj_units(i + 1))

            # ---------------- last head: attention || W_o ------------------
            pending_wo = []

            def on_chunk(sc):
                pending_wo.extend(wo_units(sc))

            gen = attn_head(NQ - 1, on_chunk)
            done = 0
            for _ in gen:
                # drain Wo work at a rate that leaves the tail short
                budget = 10
                while pending_wo[done:] and budget > 0:
                    pending_wo[done]()
                    done += 1
                    budget -= 1
            while done < len(pending_wo):
                pending_wo[done]()
                done += 1
    nc.compile()
    return nc


def make_host_constants(S, HD=128, SC=512):
    rt = np.zeros((HD, HD), np.float32)
    half = HD // 2
    for j in range(half):
        rt[j, j + half] = 1.0       # R^T upper-right block = +I
        rt[j + half, j] = -1.0      # R^T lower-left block = -I
    ident = np.eye(P, dtype=np.float32)
    tt_idx = np.arange(P)[:, None]
    ss_idx = np.arange(P)[None, :]
    mask = (ss_idx >= tt_idx).astype(np.float32)
    bf = ml_dtypes.bfloat16
    return rt.astype(bf), ident.astype(bf), mask.astype(bf)


def make_in_maps(hidden_states, cos, sin, W_qkv, W_o, NH=16, NKV=4, HD=128):
    """Shard the full inputs into 8 per-core input maps."""
    B = hidden_states.shape[0]
    S = hidden_states.shape[1]
    n_rep = NH // NKV
    rt, ident, mask = make_host_constants(S, HD)
    bf = ml_dtypes.bfloat16
    cosT = np.ascontiguousarray(cos.T).astype(bf)
    sinT = np.ascontiguousarray(sin.T).astype(bf)
    in_maps = []
    for b in range(B):
        hidT = np.ascontiguousarray(hidden_states[b].T).astype(bf)
        for g in range(NKV):
            wq = W_qkv[g * n_rep * HD:(g + 1) * n_rep * HD]
            wk = W_qkv[NH * HD + g * HD: NH * HD + (g + 1) * HD]
            wv = W_qkv[(NH + NKV) * HD + g * HD: (NH + NKV) * HD + (g + 1) * HD]
            wsh = np.concatenate([wk, wv, wq], axis=0)
            wqkvT = np.ascontiguousarray(wsh.T).astype(bf)
            woT = np.ascontiguousarray(
                W_o[:, g * n_rep * HD:(g + 1) * n_rep * HD].T).astype(bf)
            in_maps.append({
                "hidT": hidT, "wqkvT": wqkvT, "woT": woT,
                "cosT": cosT, "sinT": sinT,
                "rT": rt, "ident": ident, "maskM": mask,
            })
    return in_maps


_NC_CACHE = {}


def kernel(hidden_states, cos, sin, W_qkv, W_o):
    hidden_states = np.asarray(hidden_states, dtype=np.float32)
    cos = np.asarray(cos, dtype=np.float32)
    sin = np.asarray(sin, dtype=np.float32)
    W_qkv = np.asarray(W_qkv, dtype=np.float32)
    W_o = np.asarray(W_o, dtype=np.float32)

    B, S, HID = hidden_states.shape
    HD = cos.shape[-1]
    NH = W_o.shape[1] // HD
    NKV = (W_qkv.shape[0] // HD - NH) // 2
    n_rep = NH // NKV

    key = (S, HID, n_rep, HD)
    if key not in _NC_CACHE:
        _NC_CACHE[key] = build_attn_nc(S=S, HID=HID, NQ=n_rep, HD=HD)
    nc = _NC_CACHE[key]

    in_maps = make_in_maps(hidden_states, cos, sin, W_qkv, W_o, NH, NKV, HD)
    res = run_bass_kernel_spmd(nc, in_maps, core_ids=list(range(B * NKV)))
    outs = [np.asarray(r["out"], dtype=np.float32) for r in res.results]
    full = np.stack(
        [np.sum(outs[b * NKV:(b + 1) * NKV], axis=0, dtype=np.float32)
         for b in range(B)], axis=0)
    return full.astype(np.float32)


# revision 9
# speedup vs baseline: 1.1986x; 1.0005x over previous
"""Self-contained Trainium2 Bass kernel for GQA attention (RoPE + causal).

Problem: hidden (B=2, S=2048, HID=2048), W_qkv (3072, 2048) = 16 Q heads +
2*4 KV heads of dim 128, RoPE, causal GQA attention, W_o (2048, 2048).

Sharding: 8 cores = (batch b in {0,1}) x (KV group g in {0..3}).  Each core
gets 4 Q heads + 1 KV head (the GQA group stays intact), computes its
partial output through the 512 matching W_o columns, and the host sums the
4 partials per batch (the unshard step).  No on-device collectives.

Layout: everything "transposed" (feature dim on SBUF partitions) so every
matmul contraction lands on the partition axis.  v3 schedule:
  - attention per head is software-pipelined one t-tile deep (PV/rowsum of
    tile tt are emitted after scores of tile tt+1) so the PE never sits in
    the scores->exp->PV latency chain,
  - attention for head i is interleaved (in PE program order) with the QKV
    projection of head i+1, and attention of the last head with the W_o
    projection, so the ScalarE exp stream hides under matmuls,
  - ScalarE runs ONLY the exp stream; psum->sbuf copies live on the Vector
    engine, the causal mask multiply on GpSimd,
  - DMA order: kv-weight slice + hidden tile per contraction step first
    (the host lays W out as [wk|wv|wq]), q weights after, W_o during
    attention.
Matmul operands are bf16; accumulation/softmax arithmetic stays f32.
"""

import math

import numpy as np
import ml_dtypes

import concourse.bass as bass
import concourse.bacc as bacc
import concourse.mybir as mybir
from concourse.tile import TileContext
from concourse.bass_utils import run_bass_kernel_spmd

F32 = mybir.dt.float32
BF16 = mybir.dt.bfloat16
AF = mybir.ActivationFunctionType

P = 128  # SBUF partitions / head dim / tile edge


def build_attn_nc(S=2048, HID=2048, NQ=4, HD=128, SC=512):
    """One NeuronCore graph: NQ query heads + 1 KV head, full sequence."""
    assert HD == P
    n_h = HID // P   # contraction tiles of the QKV projection
    n_st = S // P    # 128-row tiles of the sequence
    n_sc = S // SC   # SC-wide chunks of the sequence
    n_tc = SC // P   # t-tiles per sq chunk
    n_ec = HID // SC
    scale = 1.0 / math.sqrt(HD)

    nc = bacc.Bacc("TRN2", target_bir_lowering=False, debug=False, num_devices=8)
    hidT = nc.declare_dram_parameter("hidT", [HID, S], BF16, isOutput=False)
    # host layout: [wk | wv | wq0..wq3] so the kv slice loads first
    wqkvT = nc.declare_dram_parameter("wqkvT", [HID, (NQ + 2) * P], BF16,
                                      isOutput=False)
    woT = nc.declare_dram_parameter("woT", [NQ * HD, HID], BF16, isOutput=False)
    cosT = nc.declare_dram_parameter("cosT", [HD, S], BF16, isOutput=False)
    sinT = nc.declare_dram_parameter("sinT", [HD, S], BF16, isOutput=False)
    rT = nc.declare_dram_parameter("rT", [HD, HD], BF16, isOutput=False)
    ident = nc.declare_dram_parameter("ident", [P, P], BF16, isOutput=False)
    maskM = nc.declare_dram_parameter("maskM", [P, P], BF16, isOutput=False)
    out = nc.declare_dram_parameter("out", [S, HID], BF16, isOutput=True)

    with TileContext(nc) as tc:
        with (
            tc.tile_pool(name="const", bufs=1) as constp,
            tc.tile_pool(name="wbf", bufs=1) as wbfp,
            tc.tile_pool(name="big", bufs=1) as bigp,
            tc.tile_pool(name="raw", bufs=4) as rawp,
            tc.tile_pool(name="act", bufs=1) as actp,
            tc.tile_pool(name="es", bufs=12) as esp,
            tc.tile_pool(name="rrp", bufs=2) as rrp,
            tc.tile_pool(name="outsb", bufs=4) as outp,
            tc.tile_pool(name="ps", bufs=1, space="PSUM") as psp,
        ):
            # ---------------- DMA: kv weights + hidden first --------------
            w_bf, hid_bf = [], []
            for h in range(n_h):
                wb = wbfp.tile([P, (NQ + 2) * P], BF16, tag=f"w{h}",
                               name=f"wb{h}")
                nc.sync.dma_start(out=wb[:, :2 * P],
                                  in_=wqkvT[h * P:(h + 1) * P, :2 * P])
                w_bf.append(wb)
                hb = bigp.tile([P, S], BF16, tag=f"hid{h}", name=f"hb{h}")
                pieces = 4 if h < 2 else 2
                step = S // pieces
                for pc in range(pieces):
                    nc.sync.dma_start(
                        out=hb[:, pc * step:(pc + 1) * step],
                        in_=hidT[h * P:(h + 1) * P, pc * step:(pc + 1) * step])
                hid_bf.append(hb)
            # q weights (needed from the q0 pass onwards)
            for h in range(n_h):
                nc.sync.dma_start(out=w_bf[h][:, 2 * P:],
                                  in_=wqkvT[h * P:(h + 1) * P, 2 * P:])
            rT_sb = constp.tile([HD, HD], BF16, name="rT_sb")
            nc.sync.dma_start(out=rT_sb[:, :], in_=rT[:, :])
            id_sb = constp.tile([P, P], BF16, name="id_sb")
            nc.sync.dma_start(out=id_sb[:, :], in_=ident[:, :])
            mask_sb = constp.tile([P, P], BF16, name="mask_sb")
            nc.sync.dma_start(out=mask_sb[:, :], in_=maskM[:, :])
            cos_sb = constp.tile([HD, S], BF16, name="cos_sb")
            nc.sync.dma_start(out=cos_sb[:, :], in_=cosT[:, :])
            sin_sb = constp.tile([HD, S], BF16, name="sin_sb")
            nc.sync.dma_start(out=sin_sb[:, :], in_=sinT[:, :])
            # W_o tiles reuse the last hidden-tile slots: their DMA only
            # fires once those hid tiles are dead (after the q3 projection),
            # keeping the early HBM bandwidth for the kv-pass stream.
            wo_bf = []
            for hh in range(NQ):
                wob = bigp.tile([P, HID], BF16, tag=f"hid{n_h - NQ + hh}",
                                name=f"wob{hh}")
                nc.sync.dma_start(out=wob[:, :HID // 2],
                                  in_=woT[hh * P:(hh + 1) * P, :HID // 2])
                nc.sync.dma_start(out=wob[:, HID // 2:],
                                  in_=woT[hh * P:(hh + 1) * P, HID // 2:])
                wo_bf.append(wob)
            ones_sq = constp.tile([P, P], BF16, name="ones_sq")
            nc.gpsimd.memset(ones_sq[:, :], 1.0)

            qhat = [actp.tile([HD, S], BF16, tag=f"qhat{i}", name=f"qhat{i}")
                    for i in range(NQ)]
            khat = actp.tile([HD, S], BF16, tag="khat", name="khat")
            V_bf = actp.tile([P, S], BF16, tag="V", name="V_bf")
            ohat = [actp.tile([HD, S], BF16, tag=f"ohat{i}", name=f"ohat{i}")
                    for i in range(NQ)]

            # ---------------- kv pass (all 8 PSUM banks) ------------------
            # psum tags: A=3, s=2, o=2, r=1  (8 banks total)
            ktags = ["A", "A", "A", "r"]
            vtags = ["s", "s", "o", "o"]
            ps_k = [psp.tile([P, SC], F32, tag=ktags[c], name=f"psk{c}")
                    for c in range(n_sc)]
            ps_v = [psp.tile([P, SC], F32, tag=vtags[c], name=f"psv{c}")
                    for c in range(n_sc)]
            for h in range(n_h):
                for c in range(n_sc):
                    nc.tensor.matmul(ps_k[c][:, :], lhsT=w_bf[h][:, :P],
                                     rhs=hid_bf[h][:, c * SC:(c + 1) * SC],
                                     start=(h == 0), stop=(h == n_h - 1))
                    nc.tensor.matmul(ps_v[c][:, :], lhsT=w_bf[h][:, P:2 * P],
                                     rhs=hid_bf[h][:, c * SC:(c + 1) * SC],
                                     start=(h == 0), stop=(h == n_h - 1))

            def rope_chunk(ps, c, dest):
                """psum chunk -> raw copy -> rotate matmul -> dest (roped)."""
                csl = slice(c * SC, (c + 1) * SC)
                raw = rawp.tile([P, SC], BF16, tag="raw", name=f"rw{c}")
                nc.vector.tensor_copy(raw[:, :], ps[:, :])
                psr = psp.tile([P, SC], F32, tag="A", name=f"psr{c}")
                nc.tensor.matmul(psr[:, :], lhsT=rT_sb[:, :], rhs=raw[:, :],
                                 start=True, stop=True)
                t1 = rawp.tile([P, SC], BF16, tag="t1", name=f"t1_{c}")
                nc.vector.tensor_mul(t1[:, :], psr[:, :], sin_sb[:, csl])
                t2 = rawp.tile([P, SC], BF16, tag="t2", name=f"t2_{c}")
                nc.vector.tensor_mul(t2[:, :], raw[:, :], cos_sb[:, csl])
                nc.vector.tensor_add(dest[:, csl], t1[:, :], t2[:, :])

            # v: copy + transpose to natural (t, d) layout
            rawv = rawp.tile([P, S], BF16, tag="rawv", name="rawv", bufs=1)
            for c in range(n_sc):
                nc.vector.tensor_copy(rawv[:, c * SC:(c + 1) * SC],
                                      ps_v[c][:, :])
            # k: rope into khat (frees psum banks for q0)
            for c in range(n_sc):
                rope_chunk(ps_k[c], c, khat)
            for st in range(n_st):
                pst = psp.tile([P, P], BF16, tag="s", name=f"pvt{st}")
                nc.tensor.transpose(pst[:, :], rawv[:, st * P:(st + 1) * P],
                                    id_sb[:, :])
                nc.vector.tensor_copy(V_bf[:, st * P:(st + 1) * P], pst[:, :])

            # ---------------- emitters for interleaved streams ------------
            def proj_units(o):
                """Closures: q-head o projection + rope, chunk-major
                (reversed chunk order so attention can follow in order)."""
                units = []
                for c in reversed(range(n_sc)):
                    ps = psp.tile([P, SC], F32, tag="A", name=f"pq{o}_{c}")
                    for h in range(n_h):
                        units.append(lambda ps=ps, h=h, c=c, o=o: nc.tensor.matmul(
                            ps[:, :], lhsT=w_bf[h][:, (2 + o) * P:(3 + o) * P],
                            rhs=hid_bf[h][:, c * SC:(c + 1) * SC],
                            start=(h == 0), stop=(h == n_h - 1)))
                    units.append(lambda ps=ps, c=c, o=o: rope_chunk(
                        ps, c, qhat[o]))
                return units

            def wo_units(sc):
                """Closures: W_o projection for the n_tc s-tiles of chunk sc
                + output DMA."""
                units = []
                for st in range(sc * n_tc, (sc + 1) * n_tc):
                    for ec in range(n_ec):
                        po = psp.tile([P, SC], F32, tag="A",
                                      name=f"pw{st}_{ec}")
                        for hh in range(NQ):
                            units.append(lambda po=po, hh=hh, st=st, ec=ec:
                                nc.tensor.matmul(
                                    po[:, :],
                                    lhsT=ohat[hh][:, st * P:(st + 1) * P],
                                    rhs=wo_bf[hh][:, ec * SC:(ec + 1) * SC],
                                    start=(hh == 0), stop=(hh == NQ - 1)))

                        def fin(po=po, st=st, ec=ec):
                            ot = outp.tile([P, SC], BF16, tag="osb",
                                           name=f"osb{st}_{ec}")
                            if ec % 2 == 0:
                                nc.scalar.copy(out=ot[:, :], in_=po[:, :])
                            else:
                                nc.vector.tensor_copy(ot[:, :], po[:, :])
                            nc.sync.dma_start(
                                out=out[st * P:(st + 1) * P,
                                        ec * SC:(ec + 1) * SC],
                                in_=ot[:, :])
                        units.append(fin)
                return units

            def attn_head(q, on_chunk_done):
                """Generator: causal attention for head q, chunk-major (big
                chunks first), software-pipelined TWO t-tiles deep: PV/rowsum
                of tile tt are emitted two iterations after its scores, with
                the yield (= interleaved filler PE work) in between, so the
                scores->exp->PV latency chain never stalls the PE."""
                for sc in reversed(range(n_sc)):
                    csl = slice(sc * SC, (sc + 1) * SC)
                    n_t = n_tc * (sc + 1)
                    ps_o = psp.tile([HD, SC], F32, tag="o", name=f"po{q}_{sc}")
                    ps_r = psp.tile([P, SC], F32, tag="r", name=f"pr{q}_{sc}")
                    pipe = []

                    def pv_rowsum(tt, c0, es, last):
                        nc.tensor.matmul(ps_o[:, c0:],
                                         lhsT=V_bf[:, tt * P:(tt + 1) * P],
                                         rhs=es[:, c0:],
                                         start=(tt == 0), stop=last)
                        nc.tensor.matmul(ps_r[:, c0:], lhsT=ones_sq[:, :],
                                         rhs=es[:, c0:],
                                         start=(tt == 0), stop=last)

                    for tt in range(n_t):
                        j = tt - n_tc * sc
                        c0 = j * P if j > 0 else 0
                        ps_s = psp.tile([P, SC], F32, tag="s",
                                        name=f"pss{q}_{sc}_{tt}")
                        nc.tensor.matmul(
                            ps_s[:, c0:],
                            lhsT=khat[:, tt * P:(tt + 1) * P],
                            rhs=qhat[q][:, sc * SC + c0:(sc + 1) * SC],
                            start=True, stop=True)
                        es = esp.tile([P, SC], BF16, tag="es",
                                      name=f"es{q}_{sc}_{tt}")
                        nc.scalar.activation(es[:, c0:], ps_s[:, c0:], AF.Exp,
                                             scale=scale)
                        if j >= 0:
                            nc.gpsimd.tensor_mul(es[:, c0:c0 + P],
                                                 es[:, c0:c0 + P],
                                                 mask_sb[:, :])
                        pipe.append((tt, c0, es))
                        yield
                        if len(pipe) > 2:
                            pv_rowsum(*pipe.pop(0), last=False)
                    while pipe:
                        pv_rowsum(*pipe.pop(0), last=(not pipe))
                    rr = rrp.tile([P, SC], F32, tag="rr", name=f"rr{q}_{sc}")
                    nc.vector.reciprocal_approx_fast(out=rr[:, :],
                                                     in_=ps_r[:, :])
                    nc.vector.tensor_mul(ohat[q][:, csl], ps_o[:, :], rr[:, :])
                    on_chunk_done(sc)
                    yield

            def run_interleaved(gen, units):
                """Drive the attention generator, pacing `units` closures
                evenly across its yields."""
                done = 0
                steps = 0
                total_steps = n_sc + n_tc * n_sc * (n_sc + 1) // 2
                for _ in gen:
                    steps += 1
                    want = len(units) * steps // total_steps
                    while done < want:
                        units[done]()
                        done += 1
                while done < len(units):
                    units[done]()
                    done += 1

            # ---------------- q0 projection (straight) --------------------
            for u in proj_units(0):
                u()

            # ---------------- heads 0..2: attention || next projection ----
            for i in range(NQ - 1):
                run_interleaved(attn_head(i, lambda sc: None),
                                proj_units(i + 1))

            # ---------------- last head: attention || W_o ------------------
            pending_wo = []

            def on_chunk(sc):
                pending_wo.extend(wo_units(sc))

            gen = attn_head(NQ - 1, on_chunk)
            done = 0
            for _ in gen:
                # drain Wo work at a rate that leaves the tail short
                budget = 10
                while pending_wo[done:] and budget > 0:
                    pending_wo[done]()
                    done += 1
                    budget -= 1
            while done < len(pending_wo):
                pending_wo[done]()
                done += 1
    nc.compile()
    return nc


def make_host_constants(S, HD=128, SC=512):
    rt = np.zeros((HD, HD), np.float32)
    half = HD // 2
    for j in range(half):
        rt[j, j + half] = 1.0       # R^T upper-right block = +I
        rt[j + half, j] = -1.0      # R^T lower-left block = -I
    ident = np.eye(P, dtype=np.float32)
    tt_idx = np.arange(P)[:, None]
    ss_idx = np.arange(P)[None, :]
    mask = (ss_idx >= tt_idx).astype(np.float32)
    bf = ml_dtypes.bfloat16
    return rt.astype(bf), ident.astype(bf), mask.astype(bf)


def make_in_maps(hidden_states, cos, sin, W_qkv, W_o, NH=16, NKV=4, HD=128):
    """Shard the full inputs into 8 per-core input maps."""
    B = hidden_states.shape[0]
    S = hidden_states.shape[1]
    n_rep = NH // NKV
    rt, ident, mask = make_host_constants(S, HD)
    bf = ml_dtypes.bfloat16
    cosT = np.ascontiguousarray(cos.T).astype(bf)
    sinT = np.ascontiguousarray(sin.T).astype(bf)
    in_maps = []
    for b in range(B):
        hidT = np.ascontiguousarray(hidden_states[b].T).astype(bf)
        for g in range(NKV):
            wq = W_qkv[g * n_rep * HD:(g + 1) * n_rep * HD]
            wk = W_qkv[NH * HD + g * HD: NH * HD + (g + 1) * HD]
            wv = W_qkv[(NH + NKV) * HD + g * HD: (NH + NKV) * HD + (g + 1) * HD]
            wsh = np.concatenate([wk, wv, wq], axis=0)
            wqkvT = np.ascontiguousarray(wsh.T).astype(bf)
            woT = np.ascontiguousarray(
                W_o[:, g * n_rep * HD:(g + 1) * n_rep * HD].T).astype(bf)
            in_maps.append({
                "hidT": hidT, "wqkvT": wqkvT, "woT": woT,
                "cosT": cosT, "sinT": sinT,
                "rT": rt, "ident": ident, "maskM": mask,
            })
    return in_maps


_NC_CACHE = {}


def kernel(hidden_states, cos, sin, W_qkv, W_o):
    hidden_states = np.asarray(hidden_states, dtype=np.float32)
    cos = np.asarray(cos, dtype=np.float32)
    sin = np.asarray(sin, dtype=np.float32)
    W_qkv = np.asarray(W_qkv, dtype=np.float32)
    W_o = np.asarray(W_o, dtype=np.float32)

    B, S, HID = hidden_states.shape
    HD = cos.shape[-1]
    NH = W_o.shape[1] // HD
    NKV = (W_qkv.shape[0] // HD - NH) // 2
    n_rep = NH // NKV

    key = (S, HID, n_rep, HD)
    if key not in _NC_CACHE:
        _NC_CACHE[key] = build_attn_nc(S=S, HID=HID, NQ=n_rep, HD=HD)
    nc = _NC_CACHE[key]

    in_maps = make_in_maps(hidden_states, cos, sin, W_qkv, W_o, NH, NKV, HD)
    res = run_bass_kernel_spmd(nc, in_maps, core_ids=list(range(B * NKV)))
    outs = [np.asarray(r["out"], dtype=np.float32) for r in res.results]
    full = np.stack(
        [np.sum(outs[b * NKV:(b + 1) * NKV], axis=0, dtype=np.float32)
         for b in range(B)], axis=0)
    return full.astype(np.float32)
